# revision 6
# baseline (speedup 1.0000x reference)
"""Trainium2 Bass kernel for nn_AtteMatchLay (multi-perspective cosine matching).

Math (per flattened row n, perspective p, w2 = weight**2):
    dot[n,p] = sum_d r[n,d]*m[n,d]*w2[p,d]
    n1s[n,p] = sum_d r[n,d]^2 * w2[p,d]
    n2s[n,p] = sum_d m[n,d]^2 * w2[p,d]
    cos[n,p] = dot * rsqrt(n1s) * rsqrt(n2s)

Strategy: data-parallel over the flattened N=16*512=8192 rows across 8
cores (1024 rows each). Per core the work is split into 12 (row-group,
d-block) halves; the host packs each half as one fully-contiguous 256KB
DMA. Group 1's six halves stream first, so its entire accumulation,
epilogue and output DMA complete mid-kernel and only group 0 contributes
to the tail. Per half: one fused DVE tensor_tensor computes rm|mm (the m
operand broadcast via a stride-0 dim), ACT squares r, and the PE runs
dot/n1/n2 matmuls accumulating into 6 PSUM banks. Seven warmup matmuls
on a zeroed tile keep the TensorEngine busy from t~7us so its p-state
(0.65 -> 1.2 -> 2.4 GHz with ~3us of continuous execution) is fully
ramped before the real matmul stream begins. Group tails are ordered
[dot, n1, n2] so rsqrt(n1s) and dot*rsqrt(n1s) hide under the remaining
matmuls; outputs are written bf16 (cos values ~0.05, quantization adds
~0.2% rms vs the 2e-2 gate) and upcast on host.
"""

import sys

if "/opt/trn_rl_repo" not in sys.path:
    sys.path.insert(0, "/opt/trn_rl_repo")

import numpy as np

# ---- problem constants (hardcoded per contract) ----
BSZ, SL, D, MP = 16, 512, 768, 20
N = BSZ * SL           # 8192 flattened rows
NCORES = 8
NSH = N // NCORES      # 1024 rows per core
P = 128                # SBUF partitions
NB = D // P            # 6 d-blocks
G = 2                  # row groups per core (fp32 matmul free dim <= 512)
GN = NSH // G          # 512
HALVES = [(1, b) for b in range(NB)] + [(0, b) for b in range(NB)]
NWARM = 7              # PE p-state warmup matmuls

_CACHE = {}


def _build():
    import concourse.tile as tile
    from concourse import bacc, mybir

    f32 = mybir.dt.float32
    bf16 = mybir.dt.bfloat16
    nc = bacc.Bacc(None, target_bir_lowering=False)

    # x[h*P + p, t*GN + n] = (r if t==0 else m)[g*GN + n, b*128 + p]
    # for h = HALVES.index((g, b)); each h is one contiguous 256KB DMA.
    x = nc.dram_tensor("x", [len(HALVES) * P, 2 * GN], bf16, kind="ExternalInput")
    # w2p[p, b*MP + q] = w2[q, b*128+p]
    w2p = nc.dram_tensor("w2p", [P, NB * MP], bf16, kind="ExternalInput")
    out = nc.dram_tensor("out", [MP, NSH], bf16, kind="ExternalOutput")

    SQ = mybir.ActivationFunctionType.Square
    ARSQRT = mybir.ActivationFunctionType.Abs_reciprocal_sqrt
    MUL = mybir.AluOpType.mult

    with tile.TileContext(nc) as tc:
        with (
            tc.tile_pool(name="const", bufs=1) as const,
            tc.tile_pool(name="inp", bufs=1) as inp,
            tc.tile_pool(name="prod", bufs=3) as prod,
            tc.tile_pool(name="epi", bufs=2) as epi,
            tc.tile_pool(name="psum", bufs=1, space="PSUM") as psum,
        ):
            # w2 on the ACT HWDGE queue so SP's first input DMA issues at t=0.
            w2_sb = const.tile([P, NB * MP], bf16, tag="w2")
            nc.scalar.dma_start(out=w2_sb[:], in_=w2p[:, :])

            # Dummy activation: loads the ARSQRT act table early (off the
            # critical path) instead of mid-epilogue.
            dum_i = const.tile([1, 8], f32, tag="dum_i")
            dum_o = const.tile([1, 8], f32, tag="dum_o")
            nc.gpsimd.memset(dum_i[:], 1.0)
            nc.scalar.activation(dum_o[:], dum_i[:], ARSQRT)

            # PE warmup: matmuls over a zeroed tile ramp the tensor-engine
            # p-state to 2.4GHz while the first input DMAs are in flight.
            zt = const.tile([P, MP + GN], bf16, tag="zt")
            nc.gpsimd.memset(zt[:], 0.0)
            warm_ps = psum.tile([MP, GN], f32, name="warm", tag="warm")
            for _ in range(NWARM):
                nc.tensor.matmul(
                    warm_ps[:], zt[:, 0:MP], zt[:, MP : MP + GN],
                    start=True, stop=True,
                )

            # Input: one DMA per (group, block) half, linear src, 2KB/partition.
            xsb = inp.tile([P, NB, 2, NSH], bf16, tag="x")
            for h, (g, b) in enumerate(HALVES):
                nc.sync.dma_start(
                    out=xsb[:, b, :, g * GN : (g + 1) * GN],
                    in_=x[h * P : (h + 1) * P, :],
                )

            dot_ps = [psum.tile([MP, GN], f32, name=f"dot{g}", tag=f"dot{g}") for g in range(G)]
            n1_ps = [psum.tile([MP, GN], f32, name=f"n1{g}", tag=f"n1{g}") for g in range(G)]
            n2_ps = [psum.tile([MP, GN], f32, name=f"n2{g}", tag=f"n2{g}") for g in range(G)]
            isq1 = [epi.tile([MP, GN], f32, name=f"i1{g}", tag=f"i1{g}") for g in range(G)]
            isq2 = [epi.tile([MP, GN], f32, name=f"i2{g}", tag=f"i2{g}") for g in range(G)]
            d2 = [epi.tile([MP, GN], f32, name=f"d2{g}", tag=f"d2{g}") for g in range(G)]
            cos = [epi.tile([MP, GN], bf16, name=f"cos{g}", tag=f"cos{g}") for g in range(G)]

            for g, b in HALVES:
                gsl = slice(g * GN, (g + 1) * GN)
                # Fused rm|mm on DVE: in0 = (r|m), in1 = (m|m) via a
                # stride-0 broadcast dim; bf16 keeps the 2x_1p fast path.
                pm = prod.tile([P, 2, GN], bf16, tag="pm")
                in0 = xsb[:, b, :, gsl]
                in1 = xsb[:, b, 1, gsl].unsqueeze(1).broadcast_to([P, 2, GN])
                nc.vector.tensor_tensor(pm[:], in0, in1, MUL)
                rr = prod.tile([P, GN], bf16, tag="rr")
                nc.scalar.activation(rr[:], xsb[:, b, 0, gsl], SQ)

                w2b = w2_sb[:, b * MP : (b + 1) * MP]
                st, sp = b == 0, b == NB - 1
                # dot first (gated only on DVE's pm) so the PE never stalls
                # on ACT at a half boundary; n2 last so isq1/d2 hide under it.
                nc.tensor.matmul(dot_ps[g][:], w2b, pm[:, 0, :], start=st, stop=sp)
                nc.tensor.matmul(n1_ps[g][:], w2b, rr[:], start=st, stop=sp)
                nc.tensor.matmul(n2_ps[g][:], w2b, pm[:, 1, :], start=st, stop=sp)

                if sp:
                    # Group epilogue right after its last block:
                    # cos = (dot * rsqrt(n1s)) * rsqrt(n2s).
                    nc.scalar.activation(isq1[g][:], n1_ps[g][:], ARSQRT)
                    nc.scalar.activation(isq2[g][:], n2_ps[g][:], ARSQRT)
                    if g == 1:
                        # d2 must run on DVE (GPSIMD cannot read PSUM); the
                        # SBUF-only cos mul moves to GpSimd to keep DVE free.
                        nc.vector.tensor_tensor(d2[g][:], dot_ps[g][:], isq1[g][:], MUL)
                        nc.gpsimd.tensor_tensor(cos[g][:], d2[g][:], isq2[g][:], MUL)
                    else:
                        nc.vector.tensor_tensor(d2[g][:], dot_ps[g][:], isq1[g][:], MUL)
                        nc.vector.tensor_tensor(cos[g][:], d2[g][:], isq2[g][:], MUL)
                    nc.sync.dma_start(
                        out=out[:, g * GN : (g + 1) * GN], in_=cos[g][:]
                    )

    nc.finalize()
    return nc


def get_nc():
    if "nc" not in _CACHE:
        _CACHE["nc"] = _build()
    return _CACHE["nc"]


def make_in_maps(repres, max_att, weight):
    import ml_dtypes

    bf16 = ml_dtypes.bfloat16
    r = np.ascontiguousarray(repres, dtype=np.float32).reshape(N, D)
    m = np.ascontiguousarray(max_att, dtype=np.float32).reshape(N, D)
    # w2p[p, b*MP+q] = w2[q, b*128+p]
    w2 = (weight.astype(np.float32) ** 2).T  # [D, MP]
    w2p = np.ascontiguousarray(
        w2.reshape(NB, P, MP).transpose(1, 0, 2).reshape(P, NB * MP).astype(bf16)
    )
    in_maps = []
    for c in range(NCORES):
        rows = slice(c * NSH, (c + 1) * NSH)
        rT = r[rows].T.reshape(NB, P, NSH).astype(bf16)  # [b][p][n]
        mT = m[rows].T.reshape(NB, P, NSH).astype(bf16)
        xc = np.empty((len(HALVES), P, 2, GN), dtype=bf16)
        for h, (g, b) in enumerate(HALVES):
            gsl = slice(g * GN, (g + 1) * GN)
            xc[h, :, 0, :] = rT[b, :, gsl]
            xc[h, :, 1, :] = mT[b, :, gsl]
        in_maps.append(
            {
                "x": np.ascontiguousarray(xc.reshape(len(HALVES) * P, 2 * GN)),
                "w2p": w2p,
            }
        )
    return in_maps


def gather(results):
    # results: list of dicts with "out" [MP, NSH] bf16 per core -> [BSZ, SL, MP] f32
    cols = np.concatenate(
        [results[c]["out"].astype(np.float32) for c in range(NCORES)], axis=1
    )
    return np.ascontiguousarray(cols.T).reshape(BSZ, SL, MP)


def kernel(repres, max_att, weight, **kw):
    from concourse.bass_utils import run_bass_kernel_spmd

    nc = get_nc()
    in_maps = make_in_maps(repres, max_att, weight)
    res = run_bass_kernel_spmd(nc, in_maps, list(range(NCORES)))
    return gather(res.results)
